# revision 9
# baseline (speedup 1.0000x reference)
"""GCN (2-layer, log_softmax/softmax/edge-agreement-ratio) on 8 TRN2 NeuronCores.

Sharding: nodes partitioned into 8 contiguous ranges (one per core); edges
sharded by target node. Per core the edge phase is a two-level segment-sum:
  level 1: constant block-diagonal matmuls reduce groups of 8 edges (PE),
  level 2: data-dependent one-hot matmuls scatter group sums into 128-node
           output windows (PE; one-hot built on DVE via is_equal vs iota).
Gathers of source-node features use batched indirect DMA (SWDGE) from an
AllGather'ed feature table. Inter-layer node features travel via AllGather
collectives. The agreement ratio is computed with the same machinery over
a one-hot(pred) table, with edges split by label-agreement on the host.
"""

import math
import os

import numpy as np

import concourse.bacc as bacc
import concourse.bass as bass
import concourse.mybir as mybir
import concourse.tile as tile
from concourse.bass_utils import run_bass_kernel_spmd

F32 = mybir.dt.float32
BF16 = mybir.dt.bfloat16
I32 = mybir.dt.int32
U32 = mybir.dt.uint32
OP = mybir.AluOpType
ACT = mybir.ActivationFunctionType

C = 8          # cores
F_IN = 128
HID = 64
NCLS = 40


def _cd(a, b):
    return -(-a // b)


# ---------------------------------------------------------------- host prep

def _build_struct(lc, srcrows, W, S, ZR):
    """Build gather stream + level-2 one-hot values for one core's edge set.

    lc: local target node ids (sorted ascending), srcrows: table-row ids of
    the source node per edge (same order). Returns (gidx [NF,128,64] int32,
    lg [128, W*S] f32, NF).
    """
    SHPc = W * 128
    NG = W * S * 128
    NF = _cd(NG, 1024)
    NEpad = NF * 8192

    counts = np.bincount(lc, minlength=SHPc).astype(np.int64)
    m8 = -(-counts // 8)
    m8r = m8.reshape(W, 128)
    csr = np.cumsum(m8r, axis=1)
    gstart = (S * 128 * np.arange(W, dtype=np.int64))[:, None] + csr - m8r
    gstart_flat = gstart.reshape(SHPc)

    estart = np.zeros(SHPc, np.int64)
    estart[1:] = np.cumsum(counts)[:-1]
    rank = np.arange(len(lc), dtype=np.int64) - estart[lc]
    slot = gstart_flat[lc] * 8 + rank

    stream = np.full(NEpad, ZR, np.int32)
    stream[slot] = srcrows
    gidx = (stream.reshape(NF, 8, 8, 128).transpose(0, 3, 2, 1)
            .reshape(NF, 128, 64).copy())

    total_real = int(m8.sum())
    cm = np.zeros(SHPc, np.int64)
    cm[1:] = np.cumsum(m8)[:-1]
    within = np.arange(total_real, dtype=np.int64) - np.repeat(cm, m8)
    pos = np.repeat(gstart_flat, m8) + within
    nodes = np.repeat(np.arange(SHPc, dtype=np.int64), m8)
    lg = np.full(NG, -1.0, np.float32)
    lg[pos] = (nodes % 128).astype(np.float32)
    lg_sb = np.ascontiguousarray(lg.reshape(W * S, 128).T)
    return gidx, lg_sb, NF


def _need_S(lc, W):
    counts = np.bincount(lc, minlength=W * 128).astype(np.int64)
    m8 = -(-counts // 8)
    gw = m8.reshape(W, 128).sum(1)
    return int(_cd(int(gw.max()) if len(gw) else 1, 128)) or 1


def prepare(x, edge_index, y, W1, b1, W2, b2):
    N = x.shape[0]
    SH = N // C
    W = _cd(SH, 128)
    SHP = W * 128
    NT = C * SHP
    PAD_P = SH - 128 * (W - 1)      # valid rows in last window
    assert SHP > SH, "need at least one pad row per shard for the zero row"
    ZR = SHP - 1                    # core-0 pad row: all-zero feature row

    row = np.asarray(edge_index[0], np.int64)
    col = np.asarray(edge_index[1], np.int64)
    yv = np.asarray(y, np.int64)
    E = len(row)

    def trow(n):
        return (SHP * (n // SH) + (n % SH)).astype(np.int32)

    # degrees include the self loop
    deg = np.bincount(col, minlength=N) + 1
    dinv = (1.0 / np.sqrt(deg)).astype(np.float32)

    # edges + self loops, sorted by target
    rows_a = np.concatenate([row, np.arange(N, dtype=np.int64)])
    cols_a = np.concatenate([col, np.arange(N, dtype=np.int64)])
    o = np.argsort(cols_a, kind="stable")
    rows_a, cols_a = rows_a[o], cols_a[o]

    # original edges sorted by target (for the ratio phase)
    o2 = np.argsort(col, kind="stable")
    row_r, col_r = row[o2], col[o2]
    lab_r = (yv[row_r] == yv[col_r])

    core_main, core_r1, core_r0 = [], [], []
    S = S1 = S0 = 1
    bounds = np.searchsorted(cols_a, np.arange(C + 1) * SH)
    bounds_r = np.searchsorted(col_r, np.arange(C + 1) * SH)
    for i in range(C):
        a, b = bounds[i], bounds[i + 1]
        lc = (cols_a[a:b] - i * SH).astype(np.int64)
        sr = trow(rows_a[a:b])
        core_main.append((lc, sr))
        S = max(S, _need_S(lc, W))
        a, b = bounds_r[i], bounds_r[i + 1]
        lcr = (col_r[a:b] - i * SH).astype(np.int64)
        srr = trow(row_r[a:b])
        lr = lab_r[a:b]
        core_r1.append((lcr[lr], srr[lr]))
        core_r0.append((lcr[~lr], srr[~lr]))
        S1 = max(S1, _need_S(lcr[lr], W))
        S0 = max(S0, _need_S(lcr[~lr], W))

    E1 = sum(len(t[0]) for t in core_r1)
    E0 = sum(len(t[0]) for t in core_r0)

    # constants (replicated)
    W1T = np.ascontiguousarray(np.asarray(W1, np.float32).T)          # [128,64]
    W2T = np.ascontiguousarray(np.asarray(W2, np.float32).T)          # [64,40]
    b1bc = np.tile(np.asarray(b1, np.float32)[None, :], (128, 1))     # [128,64]
    b2bc = np.tile(np.asarray(b2, np.float32)[None, :], (128, 1))     # [128,40]
    IOTA = np.tile(np.arange(128, dtype=np.float32)[None, :], (128, 1))
    BD8 = np.zeros((128, 8 * 128), np.float32)
    e = np.arange(128)
    for j in range(8):
        BD8[e, j * 128 + 16 * j + e // 8] = 1.0
    I128 = np.eye(128, dtype=np.float32)

    in_maps = []
    meta = dict(N=N, SH=SH, W=W, SHP=SHP, NT=NT, PAD_P=PAD_P, ZR=ZR,
                S=S, S1=S1, S0=S0, E=E, E1=E1, E0=E0)
    for i in range(C):
        lc, sr = core_main[i]
        gidx, lg, NF = _build_struct(lc, sr, W, S, ZR)
        g1, lg1, NF1 = _build_struct(*core_r1[i], W, S1, ZR)
        g0, lg0, NF0 = _build_struct(*core_r0[i], W, S0, ZR)
        meta["NF"], meta["NF1"], meta["NF0"] = NF, NF1, NF0

        xs = np.zeros((128, SHP), np.float32)
        xs[:, :SH] = np.asarray(x[i * SH:(i + 1) * SH], np.float32).T
        dvfull = np.zeros(SHP, np.float32)
        dvfull[:SH] = dinv[i * SH:(i + 1) * SH]
        dv = np.ascontiguousarray(dvfull.reshape(W, 128).T)

        in_maps.append({
            "xTs": xs, "gidx": gidx, "g1": g1, "g0": g0,
            "lg": lg, "lg1": lg1, "lg0": lg0, "dinv_sh": dv,
            "W1T": W1T, "W2T": W2T, "b1bc": b1bc, "b2bc": b2bc,
            "IOTA": IOTA, "BD8": BD8, "BD8B": ml_bf16(BD8), "I128": I128,
        })
    return in_maps, meta


def ml_bf16(a):
    import ml_dtypes
    return a.astype(ml_dtypes.bfloat16)


# ---------------------------------------------------------------- device build

def build(meta):
    W, SHP, NT, S, S1, S0 = (meta[k] for k in ("W", "SHP", "NT", "S", "S1", "S0"))
    NF, NF1, NF0 = meta["NF"], meta["NF1"], meta["NF0"]
    PAD_P = meta["PAD_P"]

    nc = bacc.Bacc(None, target_bir_lowering=False)

    # inputs
    xTs = nc.dram_tensor("xTs", [128, SHP], F32, kind="ExternalInput")
    gidx = nc.dram_tensor("gidx", [NF, 128, 64], I32, kind="ExternalInput")
    g1 = nc.dram_tensor("g1", [NF1, 128, 64], I32, kind="ExternalInput")
    g0 = nc.dram_tensor("g0", [NF0, 128, 64], I32, kind="ExternalInput")
    lg = nc.dram_tensor("lg", [128, W * S], F32, kind="ExternalInput")
    lg1 = nc.dram_tensor("lg1", [128, W * S1], F32, kind="ExternalInput")
    lg0 = nc.dram_tensor("lg0", [128, W * S0], F32, kind="ExternalInput")
    dinv_sh = nc.dram_tensor("dinv_sh", [128, W], F32, kind="ExternalInput")
    W1T = nc.dram_tensor("W1T", [128, HID], F32, kind="ExternalInput")
    W2T = nc.dram_tensor("W2T", [HID, NCLS], F32, kind="ExternalInput")
    b1bc = nc.dram_tensor("b1bc", [128, HID], F32, kind="ExternalInput")
    b2bc = nc.dram_tensor("b2bc", [128, NCLS], F32, kind="ExternalInput")
    IOTA = nc.dram_tensor("IOTA", [128, 128], F32, kind="ExternalInput")
    BD8 = nc.dram_tensor("BD8", [128, 8 * 128], F32, kind="ExternalInput")
    BD8B = nc.dram_tensor("BD8B", [128, 8 * 128], BF16, kind="ExternalInput")
    I128 = nc.dram_tensor("I128", [128, 128], F32, kind="ExternalInput")

    # outputs
    logp_o = nc.dram_tensor("logp_o", [SHP, NCLS], F32, kind="ExternalOutput")
    p_o = nc.dram_tensor("p_o", [SHP, NCLS], F32, kind="ExternalOutput")
    racc_o = nc.dram_tensor("racc_o", [128, 2], F32, kind="ExternalOutput")

    # internal DRAM
    h1loc = nc.dram_tensor("h1loc", [SHP, HID], F32)
    h1tab = nc.dram_tensor("h1tab", [NT, HID], F32, addr_space="Shared")
    zloc = nc.dram_tensor("zloc", [SHP, NCLS], F32)
    ztab = nc.dram_tensor("ztab", [NT, NCLS], F32, addr_space="Shared")
    ploc = nc.dram_tensor("ploc", [SHP, 1], F32)
    ptab = nc.dram_tensor("ptab", [NT, 1], F32, addr_space="Shared")
    ohtab = nc.dram_tensor("ohtab", [NT, NCLS], BF16)

    groups = [list(range(C))]

    with tile.TileContext(nc) as tc:
        cpool = tc.alloc_tile_pool(name="const", bufs=1)
        pool = tc.alloc_tile_pool(name="work", bufs=3)
        spool = tc.alloc_tile_pool(name="small", bufs=4)
        fpool = tc.alloc_tile_pool(name="fills", bufs=6)
        ppool = tc.alloc_tile_pool(name="psum", bufs=2, space="PSUM")
        qpool = tc.alloc_tile_pool(name="psum2", bufs=2, space="PSUM")

        # resident constants
        bd_sb = cpool.tile([128, 8 * 128], F32)
        nc.sync.dma_start(out=bd_sb[:], in_=BD8[:, :])
        bdb_sb = cpool.tile([128, 8 * 128], BF16)
        nc.sync.dma_start(out=bdb_sb[:], in_=BD8B[:, :])
        iota_sb = cpool.tile([128, 128], F32)
        nc.sync.dma_start(out=iota_sb[:], in_=IOTA[:, :])
        lg_sb = cpool.tile([128, W * S], F32, tag="lgmain")
        nc.sync.dma_start(out=lg_sb[:], in_=lg[:, :])
        lg1_sb = cpool.tile([128, W * S1], F32, tag="lg1")
        nc.sync.dma_start(out=lg1_sb[:], in_=lg1[:, :])
        lg0_sb = cpool.tile([128, W * S0], F32, tag="lg0")
        nc.sync.dma_start(out=lg0_sb[:], in_=lg0[:, :])
        dv_sb = cpool.tile([128, W], F32)
        nc.sync.dma_start(out=dv_sb[:], in_=dinv_sh[:, :])
        w1t_sb = cpool.tile([128, HID], F32)
        nc.sync.dma_start(out=w1t_sb[:], in_=W1T[:, :])
        w2t_sb = cpool.tile([HID, NCLS], F32)
        nc.sync.dma_start(out=w2t_sb[:], in_=W2T[:, :])
        b1_sb = cpool.tile([128, HID], F32)
        nc.sync.dma_start(out=b1_sb[:], in_=b1bc[:, :])
        b2_sb = cpool.tile([128, NCLS], F32)
        nc.sync.dma_start(out=b2_sb[:], in_=b2bc[:, :])
        id_sb = cpool.tile([128, 128], F32)
        nc.sync.dma_start(out=id_sb[:], in_=I128[:, :])
        racc = cpool.tile([128, 2], F32)
        nc.vector.memset(racc[:], 0.0)

        # ---- phase 1: h1loc = dinv * (x @ W1^T) per shard node tile
        for w in range(W):
            xt = pool.tile([128, 128], F32, tag="xt")
            nc.sync.dma_start(out=xt[:], in_=xTs[:, w * 128:(w + 1) * 128])
            ph = qpool.tile([128, HID], F32, tag="aux")
            nc.tensor.matmul(out=ph[:], lhsT=xt[:], rhs=w1t_sb[:], start=True, stop=True)
            hs = pool.tile([128, HID], F32, tag="hs")
            nc.vector.tensor_scalar_mul(hs[:], ph[:], dv_sb[:, w:w + 1])
            nc.sync.dma_start(out=h1loc[w * 128:(w + 1) * 128, :], in_=hs[:])

        nc.gpsimd.collective_compute(
            "AllGather", OP.bypass, replica_groups=groups,
            ins=[h1loc.ap().opt()], outs=[h1tab.ap().opt()])

        # ---- edge phase helper
        def edge_phase(table, gsrc, nfills, lgt, Sx, width, dt, bd, epilogue):
            """Two-level segment sum over the edge stream; epilogue(w, psum_w)."""
            cur = [-1, None]  # fills are consumed in monotone order

            def make_fill(b):
                it = spool.tile([128, 64], I32, tag="idx" + str(width))
                nc.sync.dma_start(out=it[:], in_=gsrc[b, :, :])
                gt = pool.tile([128, 64, width], dt, tag="gath")
                for u in range(64):
                    nc.gpsimd.indirect_dma_start(
                        out=gt[:, u, :], out_offset=None,
                        in_=table[:, :],
                        in_offset=bass.IndirectOffsetOnAxis(ap=it[:, u:u + 1], axis=0))
                pf = ppool.tile([128, 8 * width], F32, tag="pfill")
                for j in range(8):
                    nc.tensor.matmul(
                        out=pf[:], lhsT=bd[:, j * 128:(j + 1) * 128],
                        rhs=gt[:, j * 8:(j + 1) * 8, :],
                        start=(j == 0), stop=(j == 7))
                t = fpool.tile([128, 8 * width], dt, tag="fcopy")
                nc.vector.tensor_copy(t[:], pf[:])
                return t

            for w in range(W):
                pw = qpool.tile([128, width], F32, tag="pw")
                for s_in in range(Sx):
                    s = w * Sx + s_in
                    b, i = s // 8, s % 8
                    if b != cur[0]:
                        cur = [b, make_fill(b)]
                    oh = pool.tile([128, 128], dt, tag="oh")
                    nc.vector.tensor_scalar(
                        out=oh[:], in0=iota_sb[:],
                        scalar1=lgt[:, s:s + 1], scalar2=None, op0=OP.is_equal)
                    nc.tensor.matmul(
                        out=pw[:], lhsT=oh[:],
                        rhs=cur[1][:, i * width:(i + 1) * width],
                        start=(s_in == 0), stop=(s_in == Sx - 1))
                epilogue(w, pw)

        # ---- layer 1 edge phase (+ fused layer-2 linear)
        def epi1(w, pw):
            hsb = pool.tile([128, HID], F32, tag="hsb")
            nc.vector.scalar_tensor_tensor(
                out=hsb[:], in0=pw[:], scalar=dv_sb[:, w:w + 1], in1=b1_sb[:],
                op0=OP.mult, op1=OP.add)
            nc.vector.tensor_scalar_max(hsb[:], hsb[:], 0.0)
            pt = qpool.tile([HID, 128], F32, tag="aux")
            nc.tensor.transpose(out=pt[:], in_=hsb[:], identity=id_sb[:])
            ht = pool.tile([HID, 128], F32, tag="ht")
            nc.vector.tensor_copy(ht[:], pt[:])
            pz = qpool.tile([128, NCLS], F32, tag="aux")
            nc.tensor.matmul(out=pz[:], lhsT=ht[:], rhs=w2t_sb[:], start=True, stop=True)
            zt = pool.tile([128, NCLS], F32, tag="zt")
            nc.vector.tensor_scalar_mul(zt[:], pz[:], dv_sb[:, w:w + 1])
            nc.sync.dma_start(out=zloc[w * 128:(w + 1) * 128, :], in_=zt[:])

        edge_phase(h1tab, gidx, NF, lg_sb, S, HID, F32, bd_sb, epi1)

        nc.gpsimd.collective_compute(
            "AllGather", OP.bypass, replica_groups=groups,
            ins=[zloc.ap().opt()], outs=[ztab.ap().opt()])

        # ---- layer 2 edge phase (+ softmax / argmax epilogue)
        def epi2(w, pw):
            xo = pool.tile([128, NCLS], F32, tag="xo")
            nc.vector.scalar_tensor_tensor(
                out=xo[:], in0=pw[:], scalar=dv_sb[:, w:w + 1], in1=b2_sb[:],
                op0=OP.mult, op1=OP.add)
            mx8 = spool.tile([128, 8], F32, tag="mx8")
            nc.vector.max(mx8[:], xo[:])
            mi8 = spool.tile([128, 8], U32, tag="mi8")
            nc.vector.max_index(mi8[:], mx8[:], xo[:])
            xm = pool.tile([128, NCLS], F32, tag="xm")
            nc.vector.tensor_scalar_sub(xm[:], xo[:], mx8[:, 0:1])
            ex = pool.tile([128, NCLS], F32, tag="ex")
            nc.scalar.activation(ex[:], xm[:], ACT.Exp)
            sm = spool.tile([128, 1], F32, tag="sm")
            nc.vector.tensor_reduce(sm[:], ex[:], mybir.AxisListType.X, OP.add)
            lsm = spool.tile([128, 1], F32, tag="lsm")
            nc.scalar.activation(lsm[:], sm[:], ACT.Ln)
            lp = pool.tile([128, NCLS], F32, tag="lp")
            nc.vector.tensor_scalar_sub(lp[:], xm[:], lsm[:, 0:1])
            nc.sync.dma_start(out=logp_o[w * 128:(w + 1) * 128, :], in_=lp[:])
            rs = spool.tile([128, 1], F32, tag="rs")
            nc.vector.reciprocal(rs[:], sm[:])
            pp = pool.tile([128, NCLS], F32, tag="pp")
            nc.vector.tensor_scalar_mul(pp[:], ex[:], rs[:, 0:1])
            nc.sync.dma_start(out=p_o[w * 128:(w + 1) * 128, :], in_=pp[:])
            pf32 = spool.tile([128, 1], F32, tag="pf32")
            nc.vector.tensor_copy(pf32[:], mi8[:, 0:1])
            if w == W - 1 and PAD_P < 128:
                nc.vector.memset(pf32[PAD_P:, :], -1.0)
            nc.sync.dma_start(out=ploc[w * 128:(w + 1) * 128, :], in_=pf32[:])

        edge_phase(ztab, gidx, NF, lg_sb, S, NCLS, F32, bd_sb, epi2)

        nc.gpsimd.collective_compute(
            "AllGather", OP.bypass, replica_groups=groups,
            ins=[ploc.ap().opt()], outs=[ptab.ap().opt()])

        # ---- build one-hot(pred) table (bf16) from the gathered preds
        for k in range(NT // 1024):
            pv = spool.tile([128, 8], F32, tag="pv")
            nc.sync.dma_start(
                out=pv[:],
                in_=ptab[k * 1024:(k + 1) * 1024, :].rearrange(
                    "(u p) one -> p (u one)", p=128))
            ohb = pool.tile([128, 8, NCLS], BF16, tag="ohb")
            for u in range(8):
                nc.vector.tensor_scalar(
                    out=ohb[:, u, :], in0=iota_sb[:, :NCLS],
                    scalar1=pv[:, u:u + 1], scalar2=None, op0=OP.is_equal)
            nc.sync.dma_start(
                out=ohtab[k * 1024:(k + 1) * 1024, :].rearrange(
                    "(u p) k -> p u k", p=128),
                in_=ohb[:, :, :])

        # local pred values for the dot products
        psh = cpool.tile([128, W], F32)
        nc.sync.dma_start(
            out=psh[:], in_=ploc[:, :].rearrange("(w p) one -> p (w one)", p=128))

        # ---- ratio passes
        def mk_repi(cidx):
            def repi(w, pw):
                ohw = pool.tile([128, NCLS], F32, tag="ohw")
                nc.vector.tensor_scalar(
                    out=ohw[:], in0=iota_sb[:, :NCLS],
                    scalar1=psh[:, w:w + 1], scalar2=None, op0=OP.is_equal)
                pr = pool.tile([128, NCLS], F32, tag="pr")
                nc.vector.tensor_tensor(out=pr[:], in0=pw[:], in1=ohw[:], op=OP.mult)
                rsum = spool.tile([128, 1], F32, tag="rsum")
                nc.vector.tensor_reduce(rsum[:], pr[:], mybir.AxisListType.X, OP.add)
                nc.vector.tensor_tensor(
                    out=racc[:, cidx:cidx + 1], in0=racc[:, cidx:cidx + 1],
                    in1=rsum[:], op=OP.add)
            return repi

        edge_phase(ohtab, g1, NF1, lg1_sb, S1, NCLS, BF16, bdb_sb, mk_repi(0))
        edge_phase(ohtab, g0, NF0, lg0_sb, S0, NCLS, BF16, bdb_sb, mk_repi(1))

        nc.sync.dma_start(out=racc_o[:, :], in_=racc[:])

        for p_ in (qpool, ppool, fpool, spool, pool, cpool):
            p_.release()

    nc.compile()
    return nc


# ---------------------------------------------------------------- entry point

def kernel(x, edge_index, y, W1, b1, W2, b2):
    in_maps, meta = prepare(x, edge_index, y, W1, b1, W2, b2)
    nc = build(meta)
    trace = bool(int(os.environ.get("GCN_TRACE", "0")))
    res = run_bass_kernel_spmd(nc, in_maps, core_ids=list(range(C)), trace=trace)
    if trace:
        print("HW exec time:", res.exec_time_ns, "ns")
        if res.instructions_and_trace:
            print("trace:", res.instructions_and_trace[1])

    N, SH = meta["N"], meta["SH"]
    logp = np.concatenate([res.results[i]["logp_o"][:SH] for i in range(C)], axis=0)
    p = np.concatenate([res.results[i]["p_o"][:SH] for i in range(C)], axis=0)
    s1 = sum(float(res.results[i]["racc_o"][:, 0].sum()) for i in range(C))
    s0 = sum(float(res.results[i]["racc_o"][:, 1].sum()) for i in range(C))
    ratio = (s1 + meta["E0"] - s0) / meta["E"]
    return logp, p, np.float32(ratio)
